# revision 30
# baseline (speedup 1.0000x reference)
"""Bilateral filter (cv2 semantics: d=9, sigmaColor=sigmaSpace=75, reflect-101
border, inscribed-circle taps, L1 color distance) on 8 Trainium2 NeuronCores.

Contract: kernel(sample=np.float32[8,1024,1024,3]) -> np.float32[8,1024,1024,3].
Data parallel: one image per core.

Algorithm (residual form, symmetric tap pairs): for each pair (t,-t) the
weight field W_t = exp(cc*(L1 color diff)^2 + ln sw) is computed once on the
union region; the product G = W_t * (I(.+t) - I(.)) serves both taps:
    acc += G|gather - G|scatter      den += W|gather + W|scatter (+1 center)
    out  = center + acc / den

Engine split (balanced DVE/ACT; GPSIMD measured ~4-5us/instr on HW and is
deliberately unused for bulk work):
  DVE  (0.52 ns/el TT, 0.26 ns/el tensor_scalar@4x): sub, channel-sum adds,
       G-mul, a couple of sign-bit-AND abs pairs, epilogue
       reciprocal_approx_fast + final mul/add.
  ACT  (0.83 ns/el): most abs, Square (folds color_coeff via scale), Exp
       (folds ln space-weight via bias) -- one activation table, no switches.
  PE:  ALL accumulation as +-identity matmuls into PSUM: acc4[4,64,16] fp32
       (3 acc channels + den) = exactly 8 PSUM banks; den's +1 via a ones
       matmul. Emission is software-pipelined (stage lags L1..L4) so each
       engine's in-order stream never waits on freshly issued work.

Layout: all tap shifts are free-dim AP offsets. Each of 128 partitions owns a
[64,16] output block (16 row-bands x 8 col-blocks) with a [3,72,24] channel-
stacked bf16 halo window; 8 column chunks cover the image. Host pre-pads
(reflect), converts to bf16 and extracts halo windows; the device does all
filtering.
"""

import os
import sys

for _p in ("/opt/trn_rl_repo", "/root/.axon_site/_ro/trn_rl_repo"):
    if os.path.isdir(_p) and _p not in sys.path:
        sys.path.insert(0, _p)

import numpy as np
import ml_dtypes

import concourse.bass as bass
import concourse.bacc as bacc
import concourse.mybir as mybir
import concourse.tile as tile
from concourse.bass_utils import run_bass_kernel_spmd

BF16 = ml_dtypes.bfloat16

# Filter constants (must match the reference).
D = 9
R = D // 2  # 4
SIGMA_COLOR = 75.0
SIGMA_SPACE = 75.0
COLOR_COEFF = -0.5 / (SIGMA_COLOR * SIGMA_COLOR)
SPACE_COEFF = -0.5 / (SIGMA_SPACE * SIGMA_SPACE)
SQRT_NEG_CC = float(np.sqrt(-COLOR_COEFF))

B, H, W, C = 8, 1024, 1024, 3
N_CORES = 8

# Device geometry: per chunk, 128 partitions = row-bands x col-blocks,
# each owning a [BR, BC] output block with a [BR+2R, BC+2R] halo window.
# 32x32 blocks minimize the halo overhead among PSUM-feasible (BR*BC=1024)
# shapes and double the contiguous inner-row length vs 64x16.
BR, BC = (32, 32) if os.environ.get("BILAT_SQBLK", "1") == "1" else (64, 16)
EY, EX = BR + 2 * R, BC + 2 * R  # 72, 24
ROW_BANDS = H // BR  # 16
COL_BLOCKS_PER_CHUNK = 128 // ROW_BANDS  # 8
CHUNKS = W // (BC * COL_BLOCKS_PER_CHUNK)  # 8
MM = 512  # one PSUM bank of fp32 = one matmul output
QROWS = MM // BC  # rows per matmul slice (32)
NQ = BR // QROWS  # matmul slices per [BR,BC] plane (2)
SY, SX = BR + R, BC + R  # 68, 20: max weight-field region (work tiles)

# Symmetric tap pairs of the inscribed-circle 9x9 stencil: (dy,dx) with
# dy>0, or dy==0 and dx>0. The center tap is implicit (handled by PE).
PAIRS = [
    (dy, dx)
    for dy in range(0, R + 1)
    for dx in range(-R, R + 1)
    if dy * dy + dx * dx <= R * R and (dy > 0 or dx > 0)
]
assert len(PAIRS) == 24

# Per-pair engine assignment, balancing DVE/ACT busy time. GPSIMD (Pool)
# measured ~4-5us per tensor_tensor instruction on HW (8x the cost model),
# so the channel-sum adds stay on DVE; abs goes to ACT except for a couple
# of pairs that keep DVE/ACT level.
def _env_set(name, default):
    v = os.environ.get(name)
    if v is None:
        return set(default)
    return {int(x) for x in v.split(",") if x != ""}

ABS_DVE = _env_set("BILAT_ABS_DVE", {1, 4, 7, 10, 13, 16, 19, 23})
ADDS_DVE = _env_set("BILAT_ADDS_DVE", set(range(24)))
SQ_ACT = os.environ.get("BILAT_SQ_ACT", "1") == "1"


def _space_weight(dy, dx):
    return float(np.exp(SPACE_COEFF * (dy * dy + dx * dx)).astype(np.float32))


def _cbc(ap3, c=C):
    """Broadcast a [P, y, x] AP along a new channel axis -> [P, c, y, x]."""
    return bass.AP(ap3.tensor, ap3.offset, [ap3.ap[0], [0, c]] + list(ap3.ap[1:]))


def _build_nc():
    """Build + compile the per-core Bass program once."""
    nc = bacc.Bacc(None, target_bir_lowering=False)
    # Register const APs for the activation bias values ln(space_weight).
    for _dy, _dx in PAIRS:
        v = float(np.log(_space_weight(_dy, _dx)))
        if (mybir.dt.float32, v) not in nc.const_aps.aps:
            t = nc.alloc_sbuf_tensor(f"const-lnsw-{_dy}-{_dx}", [128, 1], mybir.dt.float32)
            nc.gpsimd.memset(t.ap(), v)
            nc.const_aps.aps[(mybir.dt.float32, v)] = t.ap()
    nc.all_engine_barrier()
    inp = nc.declare_dram_parameter(
        "win", [CHUNKS, 128, C, EY, EX], mybir.dt.bfloat16, isOutput=False
    )
    eye_in = nc.declare_dram_parameter(
        "eye", [128, 256], mybir.dt.bfloat16, isOutput=False
    )
    outp = nc.declare_dram_parameter(
        "out", [CHUNKS, 128, C, BR, BC], mybir.dt.bfloat16, isOutput=True
    )

    bf16 = mybir.dt.bfloat16
    f32 = mybir.dt.float32
    Act = mybir.ActivationFunctionType
    Alu = mybir.AluOpType

    DB = int(os.environ.get("BILAT_DBUFS", "6"))
    AB = int(os.environ.get("BILAT_ABBUFS", "4"))
    WB = int(os.environ.get("BILAT_WBUFS", "4"))
    GB = int(os.environ.get("BILAT_GBUFS", "4"))
    with tile.TileContext(nc) as tc:
        with (
            nc.allow_low_precision(
                "bf16 pipeline: num/den accumulated in fp32 PSUM from bf16 "
                "products; validated rel-err ~3e-3"
            ),
            tc.tile_pool(name="singles", bufs=1) as singles,
            tc.tile_pool(name="img", bufs=int(os.environ.get("BILAT_IBUFS", "2"))) as img_pool,
            tc.tile_pool(name="dpool", bufs=DB) as d_pool,
            tc.tile_pool(name="abpool", bufs=AB) as ab_pool,
            tc.tile_pool(name="wpip", bufs=WB) as w_pool,
            tc.tile_pool(name="gpool", bufs=GB) as g_pool,
            tc.tile_pool(name="outp", bufs=2) as out_pool,
            tc.tile_pool(name="psum", bufs=1, space="PSUM") as psum_pool,
        ):
          eye = singles.tile([128, 256], bf16, tag="eye", name="eye")
          nc.sync.dma_start(eye[:], eye_in[:])
          eyeP = eye[:, 0:128]
          eyeN = eye[:, 128:256]
          ones = singles.tile([128, QROWS, BC], bf16, tag="ones", name="ones")
          nc.gpsimd.memset(ones[:], 1.0)

          NP = len(PAIRS)
          L1 = int(os.environ.get("BILAT_L1", "1"))   # adds lag
          L2 = int(os.environ.get("BILAT_L2", "3"))   # sq/exp lag
          L3 = int(os.environ.get("BILAT_L3", "4"))   # Q-mul lag
          L4 = int(os.environ.get("BILAT_L4", "5"))   # matmul lag

          def geom(ip):
              dy, dx = PAIRS[ip]
              return dict(
                  dy=dy, dx=dx,
                  ry=R - dy, cx=R - max(dx, 0),
                  sy=BR + dy, sx=BC + abs(dx),
                  gy=dy, gx=max(dx, 0), sx0=max(-dx, 0),
              )

          if True:
            # Software-pipelined emission over the flattened (rep, chunk,
            # pair) item list: stage k of item j is emitted at step j + Lk so
            # no engine's in-order stream waits on freshly-issued work. The
            # REP amplification flows through the same pipeline, so the
            # steady state carries across reps.
            REPS = int(os.environ.get("BILAT_REP", "1"))
            n_items = REPS * CHUNKS * NP
            state = {}
            Itile = {}
            acc = {}

            def _loc(j):
                gch, ip = divmod(j, NP)
                return gch, gch % CHUNKS, ip

            def s0(j):  # DMA (chunk start), sub [DVE], abs [ACT|DVE]
                gch, ch, ip = _loc(j)
                g = geom(ip)
                sy, sx = g["sy"], g["sx"]
                if ip == 0:
                    Itile[gch] = img_pool.tile([128, C, EY, EX], bf16, tag="I", name="I")
                    nc.sync.dma_start(Itile[gch][:], inp[ch])
                I = Itile[gch]
                dt_ = d_pool.tile([128, C, SY, SX], bf16, tag="d", name="d")
                ish = I[:, :, g["ry"] + g["dy"] : g["ry"] + g["dy"] + sy,
                        g["cx"] + g["dx"] : g["cx"] + g["dx"] + sx]
                nc.vector.tensor_sub(
                    dt_[:, :, :sy, :sx], ish,
                    I[:, :, g["ry"] : g["ry"] + sy, g["cx"] : g["cx"] + sx],
                )
                ab = ab_pool.tile([128, C, SY, SX], bf16, tag="ab", name="ab")
                if ip in ABS_DVE:
                    # bf16 |x| = clear the sign bit: tensor_scalar AND @4x.
                    nc.vector.tensor_scalar(
                        ab[:, :, :sy, :sx].bitcast(mybir.dt.uint16),
                        dt_[:, :, :sy, :sx].bitcast(mybir.dt.uint16),
                        0x7FFF, None, Alu.bitwise_and,
                    )
                else:
                    nc.scalar.activation(
                        ab[:, :, :sy, :sx], dt_[:, :, :sy, :sx], Act.Abs
                    )
                state[j] = (dt_, ab)

            def s1(j):  # channel-sum adds [Pool|DVE]
                gch, ch, ip = _loc(j)
                g = geom(ip)
                sy, sx = g["sy"], g["sx"]
                dt_, ab = state[j]
                s_a = w_pool.tile([128, SY, SX], bf16, tag="s_a", name="s_a")
                s_b = w_pool.tile([128, SY, SX], bf16, tag="s_b", name="s_b")
                eng = nc.vector if ip in ADDS_DVE else nc.gpsimd
                eng.tensor_add(
                    s_a[:, :sy, :sx], ab[:, 0, :sy, :sx], ab[:, 1, :sy, :sx]
                )
                eng.tensor_add(
                    s_b[:, :sy, :sx], s_a[:, :sy, :sx], ab[:, 2, :sy, :sx]
                )
                state[j] = (dt_, s_b)

            def s2(j):  # Square + Exp [ACT]
                gch, ch, ip = _loc(j)
                g = geom(ip)
                sy, sx = g["sy"], g["sx"]
                dt_, s_b = state[j]
                Wt = w_pool.tile([128, SY, SX], bf16, tag="Wt", name="Wt")
                if SQ_ACT:
                    nc.scalar.activation(
                        s_b[:, :sy, :sx], s_b[:, :sy, :sx], Act.Square,
                        scale=SQRT_NEG_CC,
                    )
                    nc.scalar.activation(
                        Wt[:, :sy, :sx], s_b[:, :sy, :sx], Act.Exp,
                        bias=float(np.log(_space_weight(*PAIRS[ip]))),
                        scale=-1.0,
                    )
                else:
                    nc.vector.tensor_mul(
                        s_b[:, :sy, :sx], s_b[:, :sy, :sx], s_b[:, :sy, :sx]
                    )
                    nc.scalar.activation(
                        Wt[:, :sy, :sx], s_b[:, :sy, :sx], Act.Exp,
                        bias=float(np.log(_space_weight(*PAIRS[ip]))),
                        scale=COLOR_COEFF,
                    )
                state[j] = (dt_, Wt)

            def s3(j):  # G = W*d [DVE]  (residual form)
                gch, ch, ip = _loc(j)
                g = geom(ip)
                sy, sx = g["sy"], g["sx"]
                dt_, Wt = state[j]
                G = g_pool.tile([128, C, SY, SX], bf16, tag="G", name="G")
                nc.vector.tensor_mul(
                    G[:, :, :sy, :sx], _cbc(Wt[:, :sy, :sx]), dt_[:, :, :sy, :sx]
                )
                state[j] = (Wt, G)

            def _emit_P(gch, ip, Wt, G):
                g = geom(ip)
                gy, gx, sx0 = g["gy"], g["gx"], g["sx0"]
                acc4 = acc[gch]
                first = ip == 0
                last = ip == NP - 1
                for q in range(NQ):
                    qs = slice(q * QROWS, (q + 1) * QROWS)
                    # +I group: acc += G|gather, den += W|gather + W|scatter
                    for c in range(C):
                        nc.tensor.matmul(
                            acc4[:, c, qs, :], eyeP,
                            G[:, c, gy + q * QROWS : gy + (q + 1) * QROWS, gx : gx + BC],
                            start=first, stop=False,
                        )
                    nc.tensor.matmul(
                        acc4[:, 3, qs, :], eyeP,
                        Wt[:, gy + q * QROWS : gy + (q + 1) * QROWS, gx : gx + BC],
                        start=False, stop=False,
                    )
                    nc.tensor.matmul(
                        acc4[:, 3, qs, :], eyeP,
                        Wt[:, q * QROWS : (q + 1) * QROWS, sx0 : sx0 + BC],
                        start=False, stop=last,
                    )

            def _emit_N(gch, ip, Wt, G):
                g = geom(ip)
                sx0 = g["sx0"]
                acc4 = acc[gch]
                last = ip == NP - 1
                for q in range(NQ):
                    qs = slice(q * QROWS, (q + 1) * QROWS)
                    # -I group: acc -= G|scatter
                    for c in range(C):
                        nc.tensor.matmul(
                            acc4[:, c, qs, :], eyeN,
                            G[:, c, q * QROWS : (q + 1) * QROWS, sx0 : sx0 + BC],
                            start=False, stop=last,
                        )

            PAIR2 = os.environ.get("BILAT_PAIR2", "0") == "1"
            pending = {}

            def s4(j):  # accumulation matmuls [PE] (+group open / epilogue)
                gch, ch, ip = _loc(j)
                if ip == 0:
                    # den's center +1 opens the den accumulation group; the
                    # acc channels open on this pair's gather matmuls.
                    acc[gch] = psum_pool.tile([128, 4, BR, BC], f32, tag="acc4", name="acc4")
                    for q in range(NQ):
                        nc.tensor.matmul(
                            acc[gch][:, 3, q * QROWS : (q + 1) * QROWS, :],
                            eyeP, ones[:], start=True, stop=False,
                        )
                if PAIR2 and ip % 2 == 0:
                    # Defer: pair up with the next item so the PE stream runs
                    # [P(i), P(i+1), N(i), N(i+1)] -- 1 stationary switch per
                    # pair instead of 2.
                    pending[gch] = (ip, state.pop(j))
                    return
                Wt, G = state.pop(j)
                if PAIR2:
                    ip0, (Wt0, G0) = pending.pop(gch)
                    _emit_P(gch, ip0, Wt0, G0)
                    _emit_P(gch, ip, Wt, G)
                    _emit_N(gch, ip0, Wt0, G0)
                    _emit_N(gch, ip, Wt, G)
                else:
                    _emit_P(gch, ip, Wt, G)
                    _emit_N(gch, ip, Wt, G)
                last = ip == NP - 1
                if last:
                    # Epilogue: out = center + acc / den  (acc, den fp32 PSUM;
                    # den includes the center tap via the ones matmul).
                    acc4 = acc[gch]
                    I = Itile[gch]
                    rec = out_pool.tile([128, BR, BC], f32, tag="rec", name="rec")
                    nc.vector.reciprocal_approx_fast(rec[:], acc4[:, 3])
                    macc = out_pool.tile([128, C, BR, BC], bf16, tag="macc", name="macc")
                    nc.vector.tensor_mul(macc[:], acc4[:, 0:3], _cbc(rec[:]))
                    ot = out_pool.tile([128, C, BR, BC], bf16, tag="ot", name="ot")
                    nc.vector.tensor_add(
                        ot[:], macc[:], I[:, :, R : R + BR, R : R + BC]
                    )
                    nc.sync.dma_start(outp[ch], ot[:])
                    del acc[gch], Itile[gch]

            stages = [(0, s0), (L1, s1), (L2, s2), (L3, s3), (L4, s4)]
            if os.environ.get("BILAT_REVORD", "1") == "1":
                stages = list(reversed(stages))
            for step in range(n_items + L4):
                for lag, fn in stages:
                    jj = step - lag
                    if 0 <= jj < n_items:
                        fn(jj)

    nc.compile()
    return nc


_NC_CACHE = {}


def _get_nc():
    if "nc" not in _NC_CACHE:
        _NC_CACHE["nc"] = _build_nc()
    return _NC_CACHE["nc"]


def _eye_input():
    e = np.zeros((128, 256), dtype=np.float32)
    e[:, :128] = np.eye(128, dtype=np.float32)
    e[:, 128:] = -np.eye(128, dtype=np.float32)
    return e.astype(BF16)


def _prep_core_input(img):
    """[H,W,C] f32 -> [CHUNKS, 128, C, EY, EX] bf16 halo windows."""
    padded = np.pad(img, ((R, R), (R, R), (0, 0)), mode="reflect")
    padded = np.ascontiguousarray(padded.transpose(2, 0, 1)).astype(BF16)  # [C,1032,1032]
    sw = np.lib.stride_tricks.sliding_window_view(padded, (EY, EX), axis=(1, 2))
    # sw[c, y0, x0] = padded[c, y0:y0+EY, x0:x0+EX]
    wins = sw[:, :: BR, :: BC]  # [C, ROW_BANDS, W//BC, EY, EX]
    wins = wins.reshape(C, ROW_BANDS, CHUNKS, COL_BLOCKS_PER_CHUNK, EY, EX)
    wins = wins.transpose(2, 1, 3, 0, 4, 5)  # [CHUNKS, 16, 8, C, EY, EX]
    return np.ascontiguousarray(wins).reshape(CHUNKS, 128, C, EY, EX)


def _assemble_core_output(out):
    """[CHUNKS, 128, C, BR, BC] bf16 -> [H,W,C] f32."""
    o = out.reshape(CHUNKS, ROW_BANDS, COL_BLOCKS_PER_CHUNK, C, BR, BC)
    o = o.transpose(3, 1, 4, 0, 2, 5)  # [C, 16, BR, CHUNKS, 8, BC]
    o = o.reshape(C, H, W).transpose(1, 2, 0)
    return np.ascontiguousarray(o, dtype=np.float32)


def kernel(sample):
    sample = np.asarray(sample, dtype=np.float32)
    assert sample.shape == (B, H, W, C)
    nc = _get_nc()
    eye = _eye_input()
    in_maps = [
        {"win": _prep_core_input(sample[i]), "eye": eye} for i in range(B)
    ]
    res = run_bass_kernel_spmd(nc, in_maps, list(range(N_CORES)))
    return np.stack(
        [_assemble_core_output(res.results[i]["out"]) for i in range(B)], axis=0
    )


if __name__ == "__main__":
    x = np.random.RandomState(0).rand(B, H, W, C).astype(np.float32) * 255.0
    y = kernel(x)
    print("kernel output:", y.shape, y.dtype, float(y.min()), float(y.max()))


# revision 32
# speedup vs baseline: 1.3850x; 1.3850x over previous
"""Bilateral filter (cv2 semantics: d=9, sigmaColor=sigmaSpace=75, reflect-101
border, inscribed-circle taps, L1 color distance) on 8 Trainium2 NeuronCores.

Contract: kernel(sample=np.float32[8,1024,1024,3]) -> np.float32[8,1024,1024,3].
Data parallel: one image per core.

Algorithm (residual form, symmetric tap pairs): for each pair (t,-t) the
weight field W_t = exp(cc*(L1 color diff)^2 + ln sw) is computed once on the
union region; the product G = W_t * (I(.+t) - I(.)) serves both taps:
    acc += G|gather - G|scatter      den += W|gather + W|scatter (+1 center)
    out  = center + acc / den

Engine split (balanced DVE/ACT; GPSIMD measured ~4-5us/instr on HW and is
deliberately unused for bulk work):
  DVE  (0.52 ns/el TT, 0.26 ns/el tensor_scalar@4x): sub, channel-sum adds,
       G-mul, a couple of sign-bit-AND abs pairs, epilogue
       reciprocal_approx_fast + final mul/add.
  ACT  (0.83 ns/el): most abs, Square (folds color_coeff via scale), Exp
       (folds ln space-weight via bias) -- one activation table, no switches.
  PE:  ALL accumulation as +-identity matmuls into PSUM: acc4[4,64,16] fp32
       (3 acc channels + den) = exactly 8 PSUM banks; den's +1 via a ones
       matmul. Emission is software-pipelined (stage lags L1..L4) so each
       engine's in-order stream never waits on freshly issued work.

Layout: all tap shifts are free-dim AP offsets. Each of 128 partitions owns a
[64,16] output block (16 row-bands x 8 col-blocks) with a [3,72,24] channel-
stacked bf16 halo window; 8 column chunks cover the image. Host pre-pads
(reflect), converts to bf16 and extracts halo windows; the device does all
filtering.
"""

import os
import sys

for _p in ("/opt/trn_rl_repo", "/root/.axon_site/_ro/trn_rl_repo"):
    if os.path.isdir(_p) and _p not in sys.path:
        sys.path.insert(0, _p)

import numpy as np
import ml_dtypes

import concourse.bass as bass
import concourse.bacc as bacc
import concourse.mybir as mybir
import concourse.tile as tile
from concourse.bass_utils import run_bass_kernel_spmd

BF16 = ml_dtypes.bfloat16

# Filter constants (must match the reference).
D = 9
R = D // 2  # 4
SIGMA_COLOR = 75.0
SIGMA_SPACE = 75.0
COLOR_COEFF = -0.5 / (SIGMA_COLOR * SIGMA_COLOR)
SPACE_COEFF = -0.5 / (SIGMA_SPACE * SIGMA_SPACE)
SQRT_NEG_CC = float(np.sqrt(-COLOR_COEFF))

B, H, W, C = 8, 1024, 1024, 3
N_CORES = 8

# Device geometry: per chunk, 128 partitions = row-bands x col-blocks,
# each owning a [BR, BC] output block with a [BR+2R, BC+2R] halo window.
# 64x16 measured fastest on HW (32x32 has ~2% less halo work and won in sim,
# but lost 27% on HW -- the schedule is tuned to the 64x16 shapes).
BR, BC = (32, 32) if os.environ.get("BILAT_SQBLK", "0") == "1" else (64, 16)
EY, EX = BR + 2 * R, BC + 2 * R  # 72, 24
ROW_BANDS = H // BR  # 16
COL_BLOCKS_PER_CHUNK = 128 // ROW_BANDS  # 8
CHUNKS = W // (BC * COL_BLOCKS_PER_CHUNK)  # 8
MM = 512  # one PSUM bank of fp32 = one matmul output
QROWS = MM // BC  # rows per matmul slice (32)
NQ = BR // QROWS  # matmul slices per [BR,BC] plane (2)
SY, SX = BR + R, BC + R  # 68, 20: max weight-field region (work tiles)

# Symmetric tap pairs of the inscribed-circle 9x9 stencil: (dy,dx) with
# dy>0, or dy==0 and dx>0. The center tap is implicit (handled by PE).
PAIRS = [
    (dy, dx)
    for dy in range(0, R + 1)
    for dx in range(-R, R + 1)
    if dy * dy + dx * dx <= R * R and (dy > 0 or dx > 0)
]
assert len(PAIRS) == 24

# Per-pair engine assignment, balancing DVE/ACT busy time. GPSIMD (Pool)
# measured ~4-5us per tensor_tensor instruction on HW (8x the cost model),
# so the channel-sum adds stay on DVE; abs goes to ACT except for a couple
# of pairs that keep DVE/ACT level.
def _env_set(name, default):
    v = os.environ.get(name)
    if v is None:
        return set(default)
    return {int(x) for x in v.split(",") if x != ""}

ABS_DVE = _env_set("BILAT_ABS_DVE", {1, 4, 7, 10, 13, 16, 19, 23})
ADDS_DVE = _env_set("BILAT_ADDS_DVE", set(range(24)))
SQ_ACT = os.environ.get("BILAT_SQ_ACT", "1") == "1"


def _space_weight(dy, dx):
    return float(np.exp(SPACE_COEFF * (dy * dy + dx * dx)).astype(np.float32))


def _cbc(ap3, c=C):
    """Broadcast a [P, y, x] AP along a new channel axis -> [P, c, y, x]."""
    return bass.AP(ap3.tensor, ap3.offset, [ap3.ap[0], [0, c]] + list(ap3.ap[1:]))


def _build_nc():
    """Build + compile the per-core Bass program once."""
    nc = bacc.Bacc(None, target_bir_lowering=False)
    # Register const APs for the activation bias values ln(space_weight).
    for _dy, _dx in PAIRS:
        v = float(np.log(_space_weight(_dy, _dx)))
        if (mybir.dt.float32, v) not in nc.const_aps.aps:
            t = nc.alloc_sbuf_tensor(f"const-lnsw-{_dy}-{_dx}", [128, 1], mybir.dt.float32)
            nc.gpsimd.memset(t.ap(), v)
            nc.const_aps.aps[(mybir.dt.float32, v)] = t.ap()
    nc.all_engine_barrier()
    inp = nc.declare_dram_parameter(
        "win", [CHUNKS, 128, C, EY, EX], mybir.dt.bfloat16, isOutput=False
    )
    eye_in = nc.declare_dram_parameter(
        "eye", [128, 256], mybir.dt.bfloat16, isOutput=False
    )
    outp = nc.declare_dram_parameter(
        "out", [CHUNKS, 128, C, BR, BC], mybir.dt.bfloat16, isOutput=True
    )

    bf16 = mybir.dt.bfloat16
    f32 = mybir.dt.float32
    Act = mybir.ActivationFunctionType
    Alu = mybir.AluOpType

    DB = int(os.environ.get("BILAT_DBUFS", "6"))
    AB = int(os.environ.get("BILAT_ABBUFS", "4"))
    WB = int(os.environ.get("BILAT_WBUFS", "4"))
    GB = int(os.environ.get("BILAT_GBUFS", "4"))
    with tile.TileContext(nc) as tc:
        with (
            nc.allow_low_precision(
                "bf16 pipeline: num/den accumulated in fp32 PSUM from bf16 "
                "products; validated rel-err ~3e-3"
            ),
            tc.tile_pool(name="singles", bufs=1) as singles,
            tc.tile_pool(name="img", bufs=int(os.environ.get("BILAT_IBUFS", "2"))) as img_pool,
            tc.tile_pool(name="dpool", bufs=DB) as d_pool,
            tc.tile_pool(name="abpool", bufs=AB) as ab_pool,
            tc.tile_pool(name="wpip", bufs=WB) as w_pool,
            tc.tile_pool(name="gpool", bufs=GB) as g_pool,
            tc.tile_pool(name="outp", bufs=2) as out_pool,
            tc.tile_pool(name="psum", bufs=1, space="PSUM") as psum_pool,
        ):
          eye = singles.tile([128, 256], bf16, tag="eye", name="eye")
          nc.sync.dma_start(eye[:], eye_in[:])
          eyeP = eye[:, 0:128]
          eyeN = eye[:, 128:256]
          ones = singles.tile([128, QROWS, BC], bf16, tag="ones", name="ones")
          nc.gpsimd.memset(ones[:], 1.0)

          NP = len(PAIRS)
          L1 = int(os.environ.get("BILAT_L1", "1"))   # adds lag
          L2 = int(os.environ.get("BILAT_L2", "3"))   # sq/exp lag
          L3 = int(os.environ.get("BILAT_L3", "4"))   # Q-mul lag
          L4 = int(os.environ.get("BILAT_L4", "5"))   # matmul lag

          def geom(ip):
              dy, dx = PAIRS[ip]
              return dict(
                  dy=dy, dx=dx,
                  ry=R - dy, cx=R - max(dx, 0),
                  sy=BR + dy, sx=BC + abs(dx),
                  gy=dy, gx=max(dx, 0), sx0=max(-dx, 0),
              )

          if True:
            # Software-pipelined emission over the flattened (rep, chunk,
            # pair) item list: stage k of item j is emitted at step j + Lk so
            # no engine's in-order stream waits on freshly-issued work. The
            # REP amplification flows through the same pipeline, so the
            # steady state carries across reps.
            REPS = int(os.environ.get("BILAT_REP", "1"))
            n_items = REPS * CHUNKS * NP
            state = {}
            Itile = {}
            acc = {}

            def _loc(j):
                gch, ip = divmod(j, NP)
                return gch, gch % CHUNKS, ip

            def _fetch(gch):
                if gch < n_items // NP and gch not in Itile:
                    Itile[gch] = img_pool.tile([128, C, EY, EX], bf16, tag="I", name="I")
                    nc.sync.dma_start(Itile[gch][:], inp[gch % CHUNKS])

            def s0(j):  # window prefetch, sub [DVE], abs [ACT|DVE]
                gch, ch, ip = _loc(j)
                g = geom(ip)
                sy, sx = g["sy"], g["sx"]
                if ip == 0:
                    # Fetch this chunk if not already prefetched (covers
                    # chunk 0), and prefetch the NEXT chunk's window so its
                    # first sub never waits on the DMA.
                    _fetch(gch)
                    _fetch(gch + 1)
                I = Itile[gch]
                dt_ = d_pool.tile([128, C, SY, SX], bf16, tag="d", name="d")
                ish = I[:, :, g["ry"] + g["dy"] : g["ry"] + g["dy"] + sy,
                        g["cx"] + g["dx"] : g["cx"] + g["dx"] + sx]
                nc.vector.tensor_sub(
                    dt_[:, :, :sy, :sx], ish,
                    I[:, :, g["ry"] : g["ry"] + sy, g["cx"] : g["cx"] + sx],
                )
                ab = ab_pool.tile([128, C, SY, SX], bf16, tag="ab", name="ab")
                if ip in ABS_DVE:
                    # bf16 |x| = clear the sign bit: tensor_scalar AND @4x.
                    nc.vector.tensor_scalar(
                        ab[:, :, :sy, :sx].bitcast(mybir.dt.uint16),
                        dt_[:, :, :sy, :sx].bitcast(mybir.dt.uint16),
                        0x7FFF, None, Alu.bitwise_and,
                    )
                else:
                    nc.scalar.activation(
                        ab[:, :, :sy, :sx], dt_[:, :, :sy, :sx], Act.Abs
                    )
                state[j] = (dt_, ab)

            def s1(j):  # channel-sum adds [Pool|DVE]
                gch, ch, ip = _loc(j)
                g = geom(ip)
                sy, sx = g["sy"], g["sx"]
                dt_, ab = state[j]
                s_a = w_pool.tile([128, SY, SX], bf16, tag="s_a", name="s_a")
                s_b = w_pool.tile([128, SY, SX], bf16, tag="s_b", name="s_b")
                eng = nc.vector if ip in ADDS_DVE else nc.gpsimd
                eng.tensor_add(
                    s_a[:, :sy, :sx], ab[:, 0, :sy, :sx], ab[:, 1, :sy, :sx]
                )
                eng.tensor_add(
                    s_b[:, :sy, :sx], s_a[:, :sy, :sx], ab[:, 2, :sy, :sx]
                )
                state[j] = (dt_, s_b)

            def s2(j):  # Square + Exp [ACT]
                gch, ch, ip = _loc(j)
                g = geom(ip)
                sy, sx = g["sy"], g["sx"]
                dt_, s_b = state[j]
                Wt = w_pool.tile([128, SY, SX], bf16, tag="Wt", name="Wt")
                if SQ_ACT:
                    nc.scalar.activation(
                        s_b[:, :sy, :sx], s_b[:, :sy, :sx], Act.Square,
                        scale=SQRT_NEG_CC,
                    )
                    nc.scalar.activation(
                        Wt[:, :sy, :sx], s_b[:, :sy, :sx], Act.Exp,
                        bias=float(np.log(_space_weight(*PAIRS[ip]))),
                        scale=-1.0,
                    )
                else:
                    nc.vector.tensor_mul(
                        s_b[:, :sy, :sx], s_b[:, :sy, :sx], s_b[:, :sy, :sx]
                    )
                    nc.scalar.activation(
                        Wt[:, :sy, :sx], s_b[:, :sy, :sx], Act.Exp,
                        bias=float(np.log(_space_weight(*PAIRS[ip]))),
                        scale=COLOR_COEFF,
                    )
                state[j] = (dt_, Wt)

            def s3(j):  # G = W*d [DVE]  (residual form)
                gch, ch, ip = _loc(j)
                g = geom(ip)
                sy, sx = g["sy"], g["sx"]
                dt_, Wt = state[j]
                G = g_pool.tile([128, C, SY, SX], bf16, tag="G", name="G")
                nc.vector.tensor_mul(
                    G[:, :, :sy, :sx], _cbc(Wt[:, :sy, :sx]), dt_[:, :, :sy, :sx]
                )
                state[j] = (Wt, G)

            def _emit_P(gch, ip, Wt, G):
                g = geom(ip)
                gy, gx, sx0 = g["gy"], g["gx"], g["sx0"]
                acc4 = acc[gch]
                first = ip == 0
                last = ip == NP - 1
                for q in range(NQ):
                    qs = slice(q * QROWS, (q + 1) * QROWS)
                    # +I group: acc += G|gather, den += W|gather + W|scatter
                    for c in range(C):
                        nc.tensor.matmul(
                            acc4[:, c, qs, :], eyeP,
                            G[:, c, gy + q * QROWS : gy + (q + 1) * QROWS, gx : gx + BC],
                            start=first, stop=False,
                        )
                    nc.tensor.matmul(
                        acc4[:, 3, qs, :], eyeP,
                        Wt[:, gy + q * QROWS : gy + (q + 1) * QROWS, gx : gx + BC],
                        start=False, stop=False,
                    )
                    nc.tensor.matmul(
                        acc4[:, 3, qs, :], eyeP,
                        Wt[:, q * QROWS : (q + 1) * QROWS, sx0 : sx0 + BC],
                        start=False, stop=last,
                    )

            def _emit_N(gch, ip, Wt, G):
                g = geom(ip)
                sx0 = g["sx0"]
                acc4 = acc[gch]
                last = ip == NP - 1
                for q in range(NQ):
                    qs = slice(q * QROWS, (q + 1) * QROWS)
                    # -I group: acc -= G|scatter
                    for c in range(C):
                        nc.tensor.matmul(
                            acc4[:, c, qs, :], eyeN,
                            G[:, c, q * QROWS : (q + 1) * QROWS, sx0 : sx0 + BC],
                            start=False, stop=last,
                        )

            PAIR2 = os.environ.get("BILAT_PAIR2", "0") == "1"
            pending = {}

            def s4(j):  # accumulation matmuls [PE] (+group open / epilogue)
                gch, ch, ip = _loc(j)
                if ip == 0:
                    # den's center +1 opens the den accumulation group; the
                    # acc channels open on this pair's gather matmuls.
                    acc[gch] = psum_pool.tile([128, 4, BR, BC], f32, tag="acc4", name="acc4")
                    for q in range(NQ):
                        nc.tensor.matmul(
                            acc[gch][:, 3, q * QROWS : (q + 1) * QROWS, :],
                            eyeP, ones[:], start=True, stop=False,
                        )
                if PAIR2 and ip % 2 == 0:
                    # Defer: pair up with the next item so the PE stream runs
                    # [P(i), P(i+1), N(i), N(i+1)] -- 1 stationary switch per
                    # pair instead of 2.
                    pending[gch] = (ip, state.pop(j))
                    return
                Wt, G = state.pop(j)
                if PAIR2:
                    ip0, (Wt0, G0) = pending.pop(gch)
                    _emit_P(gch, ip0, Wt0, G0)
                    _emit_P(gch, ip, Wt, G)
                    _emit_N(gch, ip0, Wt0, G0)
                    _emit_N(gch, ip, Wt, G)
                else:
                    _emit_P(gch, ip, Wt, G)
                    _emit_N(gch, ip, Wt, G)
                last = ip == NP - 1
                if last:
                    # Epilogue: out = center + acc / den  (acc, den fp32 PSUM;
                    # den includes the center tap via the ones matmul).
                    acc4 = acc[gch]
                    I = Itile[gch]
                    rec = out_pool.tile([128, BR, BC], f32, tag="rec", name="rec")
                    nc.vector.reciprocal_approx_fast(rec[:], acc4[:, 3])
                    macc = out_pool.tile([128, C, BR, BC], bf16, tag="macc", name="macc")
                    nc.vector.tensor_mul(macc[:], acc4[:, 0:3], _cbc(rec[:]))
                    ot = out_pool.tile([128, C, BR, BC], bf16, tag="ot", name="ot")
                    nc.vector.tensor_add(
                        ot[:], macc[:], I[:, :, R : R + BR, R : R + BC]
                    )
                    nc.sync.dma_start(outp[ch], ot[:])
                    del acc[gch], Itile[gch]

            stages = [(0, s0), (L1, s1), (L2, s2), (L3, s3), (L4, s4)]
            if os.environ.get("BILAT_REVORD", "1") == "1":
                stages = list(reversed(stages))
            for step in range(n_items + L4):
                for lag, fn in stages:
                    jj = step - lag
                    if 0 <= jj < n_items:
                        fn(jj)

    nc.compile()
    return nc


_NC_CACHE = {}


def _get_nc():
    if "nc" not in _NC_CACHE:
        _NC_CACHE["nc"] = _build_nc()
    return _NC_CACHE["nc"]


def _eye_input():
    e = np.zeros((128, 256), dtype=np.float32)
    e[:, :128] = np.eye(128, dtype=np.float32)
    e[:, 128:] = -np.eye(128, dtype=np.float32)
    return e.astype(BF16)


def _prep_core_input(img):
    """[H,W,C] f32 -> [CHUNKS, 128, C, EY, EX] bf16 halo windows."""
    padded = np.pad(img, ((R, R), (R, R), (0, 0)), mode="reflect")
    padded = np.ascontiguousarray(padded.transpose(2, 0, 1)).astype(BF16)  # [C,1032,1032]
    sw = np.lib.stride_tricks.sliding_window_view(padded, (EY, EX), axis=(1, 2))
    # sw[c, y0, x0] = padded[c, y0:y0+EY, x0:x0+EX]
    wins = sw[:, :: BR, :: BC]  # [C, ROW_BANDS, W//BC, EY, EX]
    wins = wins.reshape(C, ROW_BANDS, CHUNKS, COL_BLOCKS_PER_CHUNK, EY, EX)
    wins = wins.transpose(2, 1, 3, 0, 4, 5)  # [CHUNKS, 16, 8, C, EY, EX]
    return np.ascontiguousarray(wins).reshape(CHUNKS, 128, C, EY, EX)


def _assemble_core_output(out):
    """[CHUNKS, 128, C, BR, BC] bf16 -> [H,W,C] f32."""
    o = out.reshape(CHUNKS, ROW_BANDS, COL_BLOCKS_PER_CHUNK, C, BR, BC)
    o = o.transpose(3, 1, 4, 0, 2, 5)  # [C, 16, BR, CHUNKS, 8, BC]
    o = o.reshape(C, H, W).transpose(1, 2, 0)
    return np.ascontiguousarray(o, dtype=np.float32)


def kernel(sample):
    sample = np.asarray(sample, dtype=np.float32)
    assert sample.shape == (B, H, W, C)
    nc = _get_nc()
    eye = _eye_input()
    in_maps = [
        {"win": _prep_core_input(sample[i]), "eye": eye} for i in range(B)
    ]
    res = run_bass_kernel_spmd(nc, in_maps, list(range(N_CORES)))
    return np.stack(
        [_assemble_core_output(res.results[i]["out"]) for i in range(B)], axis=0
    )


if __name__ == "__main__":
    x = np.random.RandomState(0).rand(B, H, W, C).astype(np.float32) * 255.0
    y = kernel(x)
    print("kernel output:", y.shape, y.dtype, float(y.min()), float(y.max()))
